# revision 21
# baseline (speedup 1.0000x reference)
"""Trainium2 Bass kernel for nn_CT_loss (data-parallel over batch, 8 cores).

v4: tensor-engine-centric. Per batch:
  u = A p + b0, c = G p + g0 (A = R diag(e), G = R^T A), vt_ai = v_ai/s_a,
  d_ai = u_i - c_a vt_ai, w_a = sum_i d^2, out(b,a) = sum_pix sqrt(w) mask.
Host multiplies per-(b,a) sums by |s_a|, applies gating + normalization.

Key identity: c_a*vt_ai = alpha[a,i]*(c_a Q_A) + beta[a,i]*(c_a Q_B)
              + h[a,i]*c_a, and h*c folds into the P0 weights (c = G p + g0).
So the only per-pixel elementwise product is y = crep2 (.) [Q_A; Q_B]
(PSUM x SBUF, one DVE op); everything else is matmuls.

Per chunk of 512 pixels (32 chunks/core), channel-major rows cls*8+b:
  MM_c: [25->48]  crep2 rows (side, a, b)       from P0+ones (XP tile)
  DVE:  y = crep2 (.) XQ[0:48] -> RHSD[0:48]    (Q_A | Q_B channel blocks)
  MM_d: [89->72]  d = u' - alpha y_A - beta y_B  (y + pad + P0 + ones)
  ACT:  SQ = Square(d) -> bf16
  MM_r: [72->128] w_a rows 32*slot + a*8 + b; 4 chunks accumulate into one
        [128,512] PSUM bank via slot-shifted weight matrices (start/stop)
  per group: ACT Sqrt -> la; DVE tensor_tensor_reduce(la*mask) -> ACC[:, g]

All matmuls are plain tile_position=(0,0); engine APs all base-0
(non-zero partition bases / col-group tiling are broken on this setup).
"""
import os
import sys

import numpy as np

for _p in ("/opt/trn_rl_repo",):
    if _p not in sys.path:
        sys.path.insert(0, _p)

import concourse.bass as bass
import concourse.bacc as bacc
import concourse.tile as tile
from concourse import mybir
from concourse.bass_utils import run_bass_kernel_spmd

from ml_dtypes import bfloat16

F32 = mybir.dt.float32
BF16 = mybir.dt.bfloat16
AF = mybir.ActivationFunctionType
OP = mybir.AluOpType

B, HW = 64, 128 * 128
NCORES, BPC = 8, 8
NCH = 512                  # pixels per chunk (one PSUM bank of fp32)
NCHUNKS = HW // NCH        # 32 chunks per core
NG = NCHUNKS // 4          # 8 groups of 4 chunks

# a -> (c1, c2, qA, qB): Q channel indices per direction
QCH = {0: (1, 2, 0, 1), 1: (0, 2, 2, 3), 2: (0, 1, 4, 5)}
QA_ORDER = [QCH[a][2] for a in range(3)]   # [0, 2, 4]
QB_ORDER = [QCH[a][3] for a in range(3)]   # [1, 3, 5]

_BUILT = None
LAST = None


def _build_nc():
    nc = bacc.Bacc(None)
    xq = nc.dram_tensor("xq", [48, HW], BF16, kind="ExternalInput")
    xz = nc.dram_tensor("xz", [41, HW], BF16, kind="ExternalInput")
    mk = nc.dram_tensor("mk", [128, NG * NCH], BF16, kind="ExternalInput")
    wc_d = nc.dram_tensor("wc", [25, 48], BF16, kind="ExternalInput")
    wd_d = nc.dram_tensor("wd", [89, 72], BF16, kind="ExternalInput")
    wr_d = nc.dram_tensor("wr", [72, 512], BF16, kind="ExternalInput")
    outp = nc.dram_tensor("out", [128, NG], F32, kind="ExternalOutput")

    with tile.TileContext(nc) as tc:
        with tc.tile_pool(name="big", bufs=1) as big, \
             tc.tile_pool(name="sq", bufs=4) as sqp, \
             tc.tile_pool(name="la", bufs=2) as lap, \
             tc.tile_pool(name="lm", bufs=2) as lmp, \
             tc.tile_pool(name="pc", bufs=3, space="PSUM") as pcp, \
             tc.tile_pool(name="pd", bufs=3, space="PSUM") as pdp, \
             tc.tile_pool(name="pw", bufs=2, space="PSUM") as pwp:
            XQ = big.tile([48, HW], BF16, tag="XQ")
            nc.sync.dma_start(XQ[:], xq[:])
            XP = big.tile([25, HW], BF16, tag="XP")
            nc.scalar.dma_start(XP[:], xz[16:41, :])
            RHSD = big.tile([89, HW], BF16, tag="RHSD")
            nc.sync.dma_start(RHSD[48:89, :], xz[:])
            WT = big.tile([89, 72 + 48], BF16, tag="WT")
            nc.scalar.dma_start(WT[0:89, 0:72], wd_d[:])
            nc.scalar.dma_start(WT[0:25, 72:120], wc_d[:])
            WRT = big.tile([72, 512], BF16, tag="WRT")
            nc.scalar.dma_start(WRT[:], wr_d[:])
            MASK = big.tile([128, NG * NCH], BF16, tag="MASK")
            nc.gpsimd.dma_start(MASK[:], mk[:])
            ACC = big.tile([128, NG], F32, tag="ACC")
            TTD = big.tile([128, NCH], BF16, tag="TTD")

            Wd = WT[0:89, 0:72]
            Wc = WT[0:25, 72:120]

            w = None
            for c in range(NCHUNKS):
                g, slot = divmod(c, 4)
                cs = slice(c * NCH, (c + 1) * NCH)
                gs = slice(g * NCH, (g + 1) * NCH)

                C = pcp.tile([48, NCH], F32, tag="C")
                nc.tensor.matmul(C[:], Wc, XP[:, cs], start=True, stop=True)

                nc.vector.tensor_mul(RHSD[0:48, cs], C[:], XQ[:, cs])

                D = pdp.tile([72, NCH], F32, tag="D")
                nc.tensor.matmul(D[:], Wd, RHSD[:, cs], start=True, stop=True)

                sq = sqp.tile([72, NCH], BF16, tag="sq")
                nc.scalar.activation(sq[:], D[:], AF.Square)

                if slot == 0:
                    w = pwp.tile([128, NCH], F32, tag="W")
                nc.tensor.matmul(w[:], WRT[:, 128 * slot:128 * slot + 128],
                                 sq[:], start=(slot == 0), stop=(slot == 3))

                if slot == 3:
                    la = lap.tile([128, NCH], BF16, tag="la")
                    nc.scalar.activation(la[:], w[:], AF.Sqrt)
                    lm = lmp.tile([128, NCH], BF16, tag="lm")
                    nc.vector.tensor_mul(lm[:], la[:], MASK[:, gs])
                    nc.scalar.activation(TTD[:], lm[:], AF.Identity,
                                         accum_out=ACC[:, g:g + 1])

            nc.sync.dma_start(outp[:], ACC[:])

    nc.compile()
    return nc


def get_nc():
    global _BUILT
    if _BUILT is None:
        _BUILT = _build_nc()
    return _BUILT


def host_constants(R, T, E):
    """Per-core weight matrices (fp64 host math -> bf16)."""
    wc = np.zeros((NCORES, 25, 48), np.float64)
    wd = np.zeros((NCORES, 89, 72), np.float64)
    wr = np.zeros((NCORES, 72, 512), np.float64)
    sabs = np.zeros((B, 3), np.float64)
    for gb in range(B):
        k, b = divmod(gb, BPC)
        Rb = R[gb].astype(np.float64)
        tb = T[gb].astype(np.float64)
        eb = E[gb].astype(np.float64)
        A = Rb * eb[None, :]
        b0 = tb - 0.5 * (Rb @ eb)
        G = Rb.T @ A
        g0 = Rb.T @ b0
        s = Rb.T @ tb
        for a in range(3):
            c1, c2, _, _ = QCH[a]
            sh = np.sign(s[a]) * max(abs(s[a]), 1e-12) if s[a] != 0 else 1e-12
            sabs[gb, a] = abs(sh)
            # crep2 columns: side*24 + a*8 + b
            for side in range(2):
                r2 = side * 24 + a * 8 + b
                for j in range(3):
                    wc[k, j * 8 + b, r2] = G[a, j]
                wc[k, 24, r2] = g0[a]
            for i in range(3):
                r = (3 * a + i) * 8 + b
                al = A[i, c1] / sh
                be = A[i, c2] / sh
                h = (tb[i] - 0.5 * (A[i, c1] + A[i, c2])) / sh
                wd[k, a * 8 + b, r] = -al
                wd[k, 24 + a * 8 + b, r] = -be
                for j in range(3):
                    wd[k, 64 + j * 8 + b, r] = A[i, j] - h * G[a, j]
                wd[k, 88, r] = b0[i] - h * g0[a]
                for slot in range(4):
                    wr[k, r, 128 * slot + 32 * slot + a * 8 + b] = 1.0
    return (wc.astype(bfloat16), wd.astype(bfloat16),
            wr.astype(bfloat16), sabs)


def make_in_maps(P0, Q0, M, wc, wd, wr):
    in_maps = []
    for k in range(NCORES):
        sl = slice(k * BPC, (k + 1) * BPC)
        # xq rows: 0..23 Q_A channels [0,2,4] (block a), 24..47 Q_B [1,3,5]
        q = Q0[sl].reshape(BPC, 6, HW)
        xq = np.empty((48, HW), np.float32)
        xq[0:24] = q[:, QA_ORDER].transpose(1, 0, 2).reshape(24, HW)
        xq[24:48] = q[:, QB_ORDER].transpose(1, 0, 2).reshape(24, HW)
        # xz: rows 0..15 zero (RHSD pad), 16..39 P0 rows j*8+b, 40 ones
        xz = np.zeros((41, HW), np.float32)
        xz[16:40] = P0[sl].reshape(BPC, 3, HW).transpose(1, 0, 2).reshape(24, HW)
        xz[40] = 1.0
        # mask rows 32*slot + a*8 + b, cols g*NCH+p <- M[b,a,(4g+slot)*NCH+p]
        mkc = np.zeros((4, 32, NG, NCH), np.float32)  # slot, row, g, p
        msl = M[sl].reshape(BPC, 3, NG, 4, NCH)       # b a g slot p
        mkc[:, 0:24] = msl.transpose(3, 1, 0, 2, 4).reshape(4, 24, NG, NCH)
        mk = mkc.reshape(128, NG * NCH)
        in_maps.append({
            "xq": xq.astype(bfloat16), "xz": xz.astype(bfloat16),
            "mk": mk.astype(bfloat16),
            "wc": np.ascontiguousarray(wc[k]),
            "wd": np.ascontiguousarray(wd[k]),
            "wr": np.ascontiguousarray(wr[k]),
        })
    return in_maps


def kernel(pred_rots, pred_P0, pred_Q0, gt_occmask, roi_extent, pred_transes):
    global LAST
    R = np.asarray(pred_rots, np.float32)
    P0 = np.asarray(pred_P0, np.float32)
    Q0 = np.asarray(pred_Q0, np.float32)
    M = np.asarray(gt_occmask, np.float32)
    E = np.asarray(roi_extent, np.float32)
    T = np.asarray(pred_transes, np.float32)

    nc = get_nc()
    wc, wd, wr, sabs = host_constants(R, T, E)
    in_maps = make_in_maps(P0, Q0, M, wc, wd, wr)
    trace = os.environ.get("KERNEL_TRACE", "0") == "1"
    LAST = run_bass_kernel_spmd(nc, in_maps, core_ids=list(range(NCORES)),
                                trace=trace)
    S = np.zeros((B, 3), np.float64)
    for k, r in enumerate(LAST.results):
        acc = r["out"].astype(np.float64).sum(axis=1).reshape(4, 32)
        rows = acc[:, 0:24].sum(axis=0)              # over slots
        S[k * BPC:(k + 1) * BPC] += rows.reshape(3, 8).T   # b, a
    S *= sabs
    Msum_a = M.sum(axis=(0, 2, 3)).astype(np.float64)  # per-a mask sums
    loss = sum(S[:, a].sum() for a in range(3) if Msum_a[a] >= 3 * B)
    total = max(Msum_a.sum(), 1.0)
    return np.asarray(np.float32(loss / total))


# revision 26
# speedup vs baseline: 1.8296x; 1.8296x over previous
"""Trainium2 Bass kernel for nn_CT_loss (data-parallel over batch, 8 cores).

v4: tensor-engine-centric. Per batch:
  u = A p + b0, c = G p + g0 (A = R diag(e), G = R^T A), vt_ai = v_ai/s_a,
  d_ai = u_i - c_a vt_ai, w_a = sum_i d^2, out(b,a) = sum_pix sqrt(w) mask.
Host multiplies per-(b,a) sums by |s_a|, applies gating + normalization.

Key identity: c_a*vt_ai = alpha[a,i]*(c_a Q_A) + beta[a,i]*(c_a Q_B)
              + h[a,i]*c_a, and h*c folds into the P0 weights (c = G p + g0).
So the only per-pixel elementwise product is y = crep2 (.) [Q_A; Q_B]
(PSUM x SBUF, one DVE op); everything else is matmuls.

Per chunk of 512 pixels (32 chunks/core), channel-major rows cls*8+b:
  MM_c: [25->48]  crep2 rows (side, a, b)       from P0+ones (XP tile)
  DVE:  y = crep2 (.) XQ[0:48] -> RHSD[0:48]    (Q_A | Q_B channel blocks)
  MM_d: [89->72]  d = u' - alpha y_A - beta y_B  (y + pad + P0 + ones)
  ACT:  SQ = Square(d) -> bf16
  MM_r: [72->128] w_a rows 32*slot + a*8 + b; 4 chunks accumulate into one
        [128,512] PSUM bank via slot-shifted weight matrices (start/stop)
  per group: ACT Sqrt -> la; DVE tensor_tensor_reduce(la*mask) -> ACC[:, g]

All matmuls are plain tile_position=(0,0); engine APs all base-0
(non-zero partition bases / col-group tiling are broken on this setup).
"""
import os
import sys

import numpy as np

for _p in ("/opt/trn_rl_repo",):
    if _p not in sys.path:
        sys.path.insert(0, _p)

import concourse.bass as bass
import concourse.bacc as bacc
import concourse.tile as tile
from concourse import mybir
from concourse.bass_utils import run_bass_kernel_spmd

from ml_dtypes import bfloat16

F32 = mybir.dt.float32
BF16 = mybir.dt.bfloat16
AF = mybir.ActivationFunctionType
OP = mybir.AluOpType

B, HW = 64, 128 * 128
NCORES, BPC = 8, 8
NCH = 512                  # pixels per chunk (one PSUM bank of fp32)
NCHUNKS = HW // NCH        # 32 chunks per core
NG = NCHUNKS // 4          # 8 groups of 4 chunks

# a -> (c1, c2, qA, qB): Q channel indices per direction
QCH = {0: (1, 2, 0, 1), 1: (0, 2, 2, 3), 2: (0, 1, 4, 5)}
QA_ORDER = [QCH[a][2] for a in range(3)]   # [0, 2, 4]
QB_ORDER = [QCH[a][3] for a in range(3)]   # [1, 3, 5]

_BUILT = None
LAST = None


def _build_nc():
    nc = bacc.Bacc(None)
    xq = nc.dram_tensor("xq", [48, HW], BF16, kind="ExternalInput")
    xz = nc.dram_tensor("xz", [25, HW], BF16, kind="ExternalInput")
    mk = nc.dram_tensor("mk", [128, NG * NCH], BF16, kind="ExternalInput")
    wc_d = nc.dram_tensor("wc", [25, 48], BF16, kind="ExternalInput")
    wd_d = nc.dram_tensor("wd", [73, 72], BF16, kind="ExternalInput")
    wr_d = nc.dram_tensor("wr", [72, 512], BF16, kind="ExternalInput")
    outp = nc.dram_tensor("out", [128, NG], F32, kind="ExternalOutput")

    with tile.TileContext(nc) as tc:
        with tc.tile_pool(name="big", bufs=1) as big, \
             tc.tile_pool(name="sq", bufs=4) as sqp, \
             tc.tile_pool(name="la", bufs=2) as lap, \
             tc.tile_pool(name="lm", bufs=2) as lmp, \
             tc.tile_pool(name="pc", bufs=3, space="PSUM") as pcp, \
             tc.tile_pool(name="pd", bufs=3, space="PSUM") as pdp, \
             tc.tile_pool(name="pw", bufs=2, space="PSUM") as pwp:
            WT = big.tile([73, 72 + 48], BF16, tag="WT")
            nc.scalar.dma_start(WT[0:73, 0:72], wd_d[:])
            nc.scalar.dma_start(WT[0:25, 72:120], wc_d[:])
            WRT = big.tile([72, 512], BF16, tag="WRT")
            nc.scalar.dma_start(WRT[:], wr_d[:])

            XQ = big.tile([48, HW], BF16, tag="XQ")
            XP = big.tile([25, HW], BF16, tag="XP")
            RHSD = big.tile([73, HW], BF16, tag="RHSD")
            MASK = big.tile([128, NG * NCH], BF16, tag="MASK")
            # column-piece loads so chunk-0 compute starts early and DMA
            # overlaps compute; alternate the two HWDGE rings
            NP = 8
            PW = HW // NP
            for p in range(NP):
                ps = slice(p * PW, (p + 1) * PW)
                e0 = nc.sync if p % 2 == 0 else nc.scalar
                e1 = nc.scalar if p % 2 == 0 else nc.sync
                e0.dma_start(XQ[:, ps], xq[:, ps])
                e1.dma_start(RHSD[48:73, ps], xz[:, ps])
                e0.dma_start(XP[:, ps], xz[:, ps])
                ms = slice(p * NCH, (p + 1) * NCH)
                e1.dma_start(MASK[:, ms], mk[:, ms])
            ACC = big.tile([128, NG], F32, tag="ACC")
            TTD = big.tile([128, NCH], BF16, tag="TTD")

            Wd = WT[0:73, 0:72]
            Wc = WT[0:25, 72:120]

            w = None
            for c in range(NCHUNKS):
                g, slot = divmod(c, 4)
                cs = slice(c * NCH, (c + 1) * NCH)
                gs = slice(g * NCH, (g + 1) * NCH)

                C = pcp.tile([48, NCH], F32, tag="C")
                nc.tensor.matmul(C[:], Wc, XP[:, cs], start=True, stop=True)

                nc.vector.tensor_mul(RHSD[0:48, cs], C[:], XQ[:, cs])

                D = pdp.tile([72, NCH], F32, tag="D")
                nc.tensor.matmul(D[:], Wd, RHSD[:, cs], start=True, stop=True)

                sq = sqp.tile([72, NCH], BF16, tag="sq")
                nc.scalar.activation(sq[:], D[:], AF.Square)

                if slot == 0:
                    w = pwp.tile([128, NCH], F32, tag="W")
                nc.tensor.matmul(w[:], WRT[:, 128 * slot:128 * slot + 128],
                                 sq[:], start=(slot == 0), stop=(slot == 3))

                if slot == 3:
                    la = lap.tile([128, NCH], BF16, tag="la")
                    nc.scalar.activation(la[:], w[:], AF.Sqrt)
                    lm = lmp.tile([128, NCH], BF16, tag="lm")
                    nc.vector.tensor_mul(lm[:], la[:], MASK[:, gs])
                    nc.scalar.activation(TTD[:], lm[:], AF.Identity,
                                         accum_out=ACC[:, g:g + 1])

            nc.sync.dma_start(outp[:], ACC[:])

    nc.compile()
    return nc


def get_nc():
    global _BUILT
    if _BUILT is None:
        _BUILT = _build_nc()
    return _BUILT


def host_constants(R, T, E):
    """Per-core weight matrices (fp64 host math -> bf16)."""
    wc = np.zeros((NCORES, 25, 48), np.float64)
    wd = np.zeros((NCORES, 73, 72), np.float64)
    wr = np.zeros((NCORES, 72, 512), np.float64)
    sabs = np.zeros((B, 3), np.float64)
    for gb in range(B):
        k, b = divmod(gb, BPC)
        Rb = R[gb].astype(np.float64)
        tb = T[gb].astype(np.float64)
        eb = E[gb].astype(np.float64)
        A = Rb * eb[None, :]
        b0 = tb - 0.5 * (Rb @ eb)
        G = Rb.T @ A
        g0 = Rb.T @ b0
        s = Rb.T @ tb
        for a in range(3):
            c1, c2, _, _ = QCH[a]
            sh = np.sign(s[a]) * max(abs(s[a]), 1e-12) if s[a] != 0 else 1e-12
            sabs[gb, a] = abs(sh)
            # crep2 columns: side*24 + a*8 + b
            for side in range(2):
                r2 = side * 24 + a * 8 + b
                for j in range(3):
                    wc[k, j * 8 + b, r2] = G[a, j]
                wc[k, 24, r2] = g0[a]
            for i in range(3):
                r = (3 * a + i) * 8 + b
                al = A[i, c1] / sh
                be = A[i, c2] / sh
                h = (tb[i] - 0.5 * (A[i, c1] + A[i, c2])) / sh
                wd[k, a * 8 + b, r] = -al
                wd[k, 24 + a * 8 + b, r] = -be
                for j in range(3):
                    wd[k, 48 + j * 8 + b, r] = A[i, j] - h * G[a, j]
                wd[k, 72, r] = b0[i] - h * g0[a]
                for slot in range(4):
                    wr[k, r, 128 * slot + 32 * slot + a * 8 + b] = 1.0
    return (wc.astype(bfloat16), wd.astype(bfloat16),
            wr.astype(bfloat16), sabs)


def make_in_maps(P0, Q0, M, wc, wd, wr):
    in_maps = []
    for k in range(NCORES):
        sl = slice(k * BPC, (k + 1) * BPC)
        # xq rows: 0..23 Q_A channels [0,2,4] (block a), 24..47 Q_B [1,3,5]
        q = Q0[sl].reshape(BPC, 6, HW)
        xq = np.empty((48, HW), np.float32)
        xq[0:24] = q[:, QA_ORDER].transpose(1, 0, 2).reshape(24, HW)
        xq[24:48] = q[:, QB_ORDER].transpose(1, 0, 2).reshape(24, HW)
        # xz: rows 0..23 P0 rows j*8+b, 24 ones
        xz = np.zeros((25, HW), np.float32)
        xz[0:24] = P0[sl].reshape(BPC, 3, HW).transpose(1, 0, 2).reshape(24, HW)
        xz[24] = 1.0
        # mask rows 32*slot + a*8 + b, cols g*NCH+p <- M[b,a,(4g+slot)*NCH+p]
        mkc = np.zeros((4, 32, NG, NCH), np.float32)  # slot, row, g, p
        msl = M[sl].reshape(BPC, 3, NG, 4, NCH)       # b a g slot p
        mkc[:, 0:24] = msl.transpose(3, 1, 0, 2, 4).reshape(4, 24, NG, NCH)
        mk = mkc.reshape(128, NG * NCH)
        in_maps.append({
            "xq": xq.astype(bfloat16), "xz": xz.astype(bfloat16),
            "mk": mk.astype(bfloat16),
            "wc": np.ascontiguousarray(wc[k]),
            "wd": np.ascontiguousarray(wd[k]),
            "wr": np.ascontiguousarray(wr[k]),
        })
    return in_maps


def kernel(pred_rots, pred_P0, pred_Q0, gt_occmask, roi_extent, pred_transes):
    global LAST
    R = np.asarray(pred_rots, np.float32)
    P0 = np.asarray(pred_P0, np.float32)
    Q0 = np.asarray(pred_Q0, np.float32)
    M = np.asarray(gt_occmask, np.float32)
    E = np.asarray(roi_extent, np.float32)
    T = np.asarray(pred_transes, np.float32)

    nc = get_nc()
    wc, wd, wr, sabs = host_constants(R, T, E)
    in_maps = make_in_maps(P0, Q0, M, wc, wd, wr)
    trace = os.environ.get("KERNEL_TRACE", "0") == "1"
    LAST = run_bass_kernel_spmd(nc, in_maps, core_ids=list(range(NCORES)),
                                trace=trace)
    S = np.zeros((B, 3), np.float64)
    for k, r in enumerate(LAST.results):
        acc = r["out"].astype(np.float64).sum(axis=1).reshape(4, 32)
        rows = acc[:, 0:24].sum(axis=0)              # over slots
        S[k * BPC:(k + 1) * BPC] += rows.reshape(3, 8).T   # b, a
    S *= sabs
    Msum_a = M.sum(axis=(0, 2, 3)).astype(np.float64)  # per-a mask sums
    loss = sum(S[:, a].sum() for a in range(3) if Msum_a[a] >= 3 * B)
    total = max(Msum_a.sum(), 1.0)
    return np.asarray(np.float32(loss / total))
